# revision 1
# baseline (speedup 1.0000x reference)
"""Trainium2 Bass kernel for batched DWT (db4, single level) via banded matmul.

Problem: x [1024, 4096] f32, W [4096, 4096] f32 wavelet analysis matrix
(transposed banded circulant built from the 8-tap db4 filter pair).
    y = x @ W;  out = concat([y[:, ::2], y[:, 1::2]], axis=1)

Key structure: W[j, n] is nonzero only for j - 2*(n//2) in [0, 8) (mod 4096).
So output columns [122*i, 122*i+122) depend only on x columns
[122*i, 122*i+128) (mod 4096), and the 128x122 coefficient block is the SAME
for every i (circulant shift invariance). Instead of a dense 4096x4096 matmul
(64 MB of W traffic per core) each core does 34 small PE matmuls against one
shared 128x122 band matrix extracted from W's top-left corner, with the
even/odd de-interleave folded into the band matrix's column order.

Sharding: pure data parallel over batch. Each of the 8 cores gets 128 rows.
The host pre-transposes its shard into the lhsT (stationary operand) tile
layout H[:, 128i:128i+128] = x_shard.T[122i : 122i+128, :] (circular pad),
with the band matrix prepended as the first 122 columns so the whole working
set arrives in a few chunked DMAs (~4.3 MB HBM traffic per core, memory-bound:
~12 us of DMA at ~360 GB/s/core vs ~7 us of PE work hidden under it).
"""

import numpy as np

import concourse.bacc as bacc
import concourse.tile as tile
from concourse import mybir
from concourse.bass_utils import run_bass_kernel_spmd

N_CORES = 8
BATCH = 1024
SEQ = 4096
R = BATCH // N_CORES          # rows per core = 128
P = 128                       # partitions
BLK = 122                     # output columns per block (122 + 6 tap halo = 128)
NBLK = 34                     # ceil(4096 / 122); last block has 70 real columns
HALF = BLK // 2               # 61 even (approx) + 61 odd (detail) cols per block
HCOLS = BLK + NBLK * P        # 122 (band matrix) + 4352 (lhsT tiles)

# chunks of blocks: (first block, n blocks). Each chunk = one input DMA,
# one output DMA; psum groups of <=4 blocks inside. Progressive sizes: small
# first chunk -> PE starts early; small last chunk -> short exposed tail store.
# (verified on HW at rel err 8.3e-08; TimelineSim 16586 ns/core)
CHUNKS = [(0, 2), (2, 5), (7, 9), (16, 9), (25, 6), (31, 3)]

FP32 = mybir.dt.float32

# tuning knobs (see _build_bass); defaults picked via TimelineSim + HW slope
OPTS = {
    "chunks": CHUNKS,
    "alt_copy": True,    # alternate deinterleave copies between DVE and ACT
    "alt_load": True,    # alternate load DMAs between the two HWDGE rings
    "mm_dtype": "f32",   # "f32" | "f32r" (bitcast matmul operands to float32r)
}

_CACHE = {}


def _build_bass(repeat=1, opts=None):
    """Build (once) the single-core Bass/Tile program; all 8 cores run it SPMD.

    repeat > 1 replicates the whole body back-to-back inside one NEFF —
    used only for benchmarking (wall-clock slope vs repeat count isolates
    per-pass HW time from host/tunnel dispatch overhead)."""
    o = dict(OPTS, **(opts or {}))
    chunks = o["chunks"]
    loop_n = o.get("loop_n", 0)  # >0: wrap body in a HW loop (bench only)
    nc = bacc.Bacc(
        "TRN2",
        target_bir_lowering=False,
        debug=False,
        enable_asserts=False,
        num_devices=N_CORES,
    )
    h_t = nc.dram_tensor("h", [P, HCOLS], FP32, kind="ExternalInput")
    out_t = nc.dram_tensor("out", [R, SEQ], FP32, kind="ExternalOutput")
    h_ap = h_t.ap()
    out_ap = out_t.ap()

    with tile.TileContext(nc) as tc:
        with (
            tc.tile_pool(name="hpool", bufs=o.get("hbufs", 4)) as hp,
            tc.tile_pool(name="opool", bufs=o.get("obufs", 4)) as op,
            tc.tile_pool(name="psum", bufs=8, space="PSUM") as psump,
        ):
            # out DRAM viewed as [p, 2 halves, 2048]: half 0 = approx, 1 = detail
            out_v = out_ap.rearrange("p (s m) -> p s m", s=2)

            def mm_ap(ap):
                if o["mm_dtype"] == "f32r":
                    return ap.bitcast(mybir.dt.float32r)
                return ap

            def emit_pass():
                btile = None
                copy_i = 0
                for ci, (b0, nb) in enumerate(chunks):
                    btile, copy_i = emit_chunk(ci, b0, nb, btile, copy_i)

            def emit_chunk(ci, b0, nb, btile, copy_i):
                # chunk 0's DMA also carries the 122-col band matrix so the
                # first matmuls need exactly one DMA wait.
                lead = BLK if b0 == 0 else 0
                dcol0 = BLK + P * b0 - lead
                ht = hp.tile([P, lead + P * nb], FP32, tag="h")
                ld_eng = nc.scalar if (o["alt_load"] and ci % 2) else nc.sync
                ld_eng.dma_start(ht[:], h_ap[:, dcol0 : BLK + P * (b0 + nb)])
                if b0 == 0:
                    btile = ht  # band matrix lives in cols [0:122] of chunk 0

                # number of real output cols this chunk contributes per half
                ceff = min(HALF * (b0 + nb), SEQ // 2) - HALF * b0
                otile = op.tile([P, 2 * ceff], FP32, tag="o")
                o_v = otile[:].rearrange("p (s m) -> p s m", s=2)

                def copy(dst, src):
                    nonlocal copy_i
                    if o["alt_copy"] and copy_i % 2:
                        nc.scalar.copy(dst, src)
                    else:
                        nc.vector.tensor_copy(dst, src)
                    copy_i += 1

                stored = 0  # chunk-local half-cols already flushed to HBM

                def flush(upto):
                    nonlocal stored
                    if upto > stored:
                        st = nc.sync if (o["alt_load"] and ci % 2) else nc.scalar
                        st.dma_start(
                            out_v[:, :, HALF * b0 + stored : HALF * b0 + upto],
                            o_v[:, :, stored:upto],
                        )
                        stored = upto

                for g0 in range(0, nb, 4):
                    gn = min(4, nb - g0)
                    ps = psump.tile([P, BLK * 4], FP32, tag="ps")
                    for q in range(gn):
                        blk = b0 + g0 + q
                        col = lead + P * (blk - b0) if b0 == 0 else P * (blk - b0)
                        nc.tensor.matmul(
                            ps[:, BLK * q : BLK * (q + 1)],
                            mm_ap(ht[:, col : col + P]),
                            mm_ap(btile[:, 0:BLK]),
                            start=True,
                            stop=True,
                        )
                    # de-interleaving PSUM -> SBUF copy. Full blocks in one
                    # 4D-AP copy; the final 70-wide block separately.
                    nfull = gn if (b0 + g0 + gn) % NBLK else gn - 1
                    loc0 = HALF * g0  # chunk-local col offset of group
                    if nfull:
                        src = ps[:, 0 : BLK * nfull].rearrange(
                            "p (g s t) -> p g s t", s=2, t=HALF
                        )
                        dst = o_v[:, :, loc0 : loc0 + HALF * nfull].rearrange(
                            "p s (g t) -> p g s t", t=HALF
                        )
                        copy(dst, src)
                    if nfull != gn:  # last block: 70 real cols = 35 + 35
                        src = ps[:, BLK * nfull : BLK * (nfull + 1)].rearrange(
                            "p (s t) -> p s t", t=HALF
                        )[:, :, 0:35]
                        dst = o_v[:, :, loc0 + HALF * nfull : loc0 + HALF * nfull + 35]
                        copy(dst, src)
                    se = o.get("store_every", 0)  # groups per intermediate store
                    if se and (g0 // 4 + 1) % se == 0 and g0 + gn < nb:
                        flush(HALF * (g0 + gn))

                flush(ceff)
                return btile, copy_i

            if loop_n:
                with tc.For_i(0, loop_n, 1):
                    emit_pass()
            else:
                for _ in range(repeat):
                    emit_pass()

    # Note: instructions that end up with >1 sync wait (walrus encodes only
    # one on fp32 LDW+MM pairs etc.) are legalized by bacc's compile() below.
    nc.compile()
    return nc


def _get_nc(repeat=1, opts=None):
    key = ("nc", repeat, repr(sorted((opts or {}).items(), key=str)))
    if key not in _CACHE:
        _CACHE[key] = _build_bass(repeat, opts)
    return _CACHE[key]


def _pack_host(x, bmat):
    """Per-core input tensors: [band matrix | lhsT tiles], where lhsT tile i
    is x_shard.T[122i : 122i+128, :] (circularly padded)."""
    hs = []
    for c in range(N_CORES):
        xs = np.ascontiguousarray(x[R * c : R * (c + 1)].T)  # [4096, 128]
        xtp = np.concatenate([xs, xs[:P]], axis=0)            # circular pad
        H = np.empty((P, HCOLS), dtype=np.float32)
        H[:, 0:BLK] = bmat
        for i in range(NBLK):
            H[:, BLK + P * i : BLK + P * (i + 1)] = xtp[BLK * i : BLK * i + P]
        hs.append(H)
    return hs


def _band_matrix(W):
    """128x122 coefficient block with de-interleaved (evens-first) columns."""
    perm = np.concatenate([np.arange(0, BLK, 2), np.arange(1, BLK, 2)])
    return np.ascontiguousarray(np.asarray(W, dtype=np.float32)[0:P, perm])


def run(x, W, trace=False):
    x = np.ascontiguousarray(np.asarray(x, dtype=np.float32))
    assert x.shape == (BATCH, SEQ), x.shape
    in_maps = [{"h": h} for h in _pack_host(x, _band_matrix(W))]
    res = run_bass_kernel_spmd(
        _get_nc(), in_maps, core_ids=list(range(N_CORES)), trace=trace
    )
    out = np.concatenate([res.results[c]["out"] for c in range(N_CORES)], axis=0)
    return out, res


def kernel(x, W):
    out, _ = run(x, W)
    return out



# revision 26
# speedup vs baseline: 1.5266x; 1.5266x over previous
"""Trainium2 Bass kernel for batched DWT (db4, single level) via banded matmul.

Problem: x [1024, 4096] f32, W [4096, 4096] f32 wavelet analysis matrix
(transposed banded circulant built from the 8-tap db4 filter pair).
    y = x @ W;  out = concat([y[:, ::2], y[:, 1::2]], axis=1)

Key structure: W[j, n] is nonzero only for j - 2*(n//2) in [0, 8) (mod 4096).
So output columns [122*i, 122*i+122) depend only on x columns
[122*i, 122*i+128) (mod 4096), and the 128x122 coefficient block is the SAME
for every i (circulant shift invariance). Instead of a dense 4096x4096 matmul
(64 MB of W traffic per core) each core does 34 small PE matmuls against one
shared 128x122 band matrix extracted from W's top-left corner, with the
even/odd de-interleave folded into the band matrix's column order.

Sharding: pure data parallel over batch. Each of the 8 cores gets 128 rows.
The host pre-transposes its shard into the lhsT (stationary operand) tile
layout H[:, 128i:128i+128] = x_shard.T[122i : 122i+128, :] (circular pad),
with the band matrix prepended as the first 122 columns so the whole working
set arrives in a few chunked DMAs (~4.3 MB HBM traffic per core, memory-bound:
~12 us of DMA at ~360 GB/s/core vs ~7 us of PE work hidden under it).
"""

import contextlib

import numpy as np

import concourse.bacc as bacc
import concourse.bass as cbass
import concourse.tile as tile
from concourse import mybir
from concourse.bass_utils import run_bass_kernel_spmd


@contextlib.contextmanager
def _skip_const_memsets():
    """Suppress the 4 const-tensor memsets Bass.__init__ emits on the Pool
    queue. They gate the prologue all-engine barrier (~0.4us before the first
    load DMA can decode) and this kernel never reads the const APs: its only
    ops are DMACopy / Matmult / DVE TensorCopy / Activation-Copy, and for
    Copy the bias/scale stay immediates (bass.py activation())."""
    owners = [k for k in cbass.BassGpSimd.__mro__ if "memset" in vars(k)]
    saved = [(k, vars(k)["memset"]) for k in owners]
    for k in owners:
        k.memset = lambda self, ap, c: None
    try:
        yield
    finally:
        for k, fn in saved:
            k.memset = fn

N_CORES = 8
BATCH = 1024
SEQ = 4096
R = BATCH // N_CORES          # rows per core = 128
P = 128                       # partitions
BLK = 122                     # output columns per block (122 + 6 tap halo = 128)
NBLK = 34                     # ceil(4096 / 122); last block has 70 real columns
HALF = BLK // 2               # 61 even (approx) + 61 odd (detail) cols per block
HCOLS = BLK + NBLK * P        # 122 (band matrix) + 4352 (lhsT tiles)

FP32 = mybir.dt.float32
FP16 = mybir.dt.float16

# tuning knobs (see _build_bass); defaults picked via TimelineSim + HW slope.
# "loads"/"stores" are block counts per input/output DMA (both sum to 34).
# All loads are emitted first on the SP queue so they stream back-to-back;
# stores follow on the same queue but can never delay a load. Copies
# (psum->sbuf fp16 deinterleave) alternate DVE/ACT. Small last load+store ->
# short exposed load->mm->copy->store tail.
OPTS = {
    "loads": [4, 8, 8, 8, 6],
    "stores": [4, 8, 8, 8, 6],
}

_CACHE = {}


def _build_bass(repeat=1, opts=None):
    """Build (once) the single-core Bass/Tile program; all 8 cores run it SPMD.

    repeat > 1 replicates the whole body back-to-back inside one NEFF —
    used only for benchmarking (wall-clock slope vs repeat count isolates
    per-pass HW time from host/tunnel dispatch overhead)."""
    o = dict(OPTS, **(opts or {}))
    loads = o["loads"]
    stores = o["stores"]
    assert sum(loads) == NBLK and sum(stores) == NBLK, (loads, stores)
    loop_n = o.get("loop_n", 0)  # >0: wrap body in a HW loop (bench only)
    with _skip_const_memsets():
        nc = bacc.Bacc(
            "TRN2",
            target_bir_lowering=False,
            debug=False,
            enable_asserts=False,
            num_devices=N_CORES,
        )
    h_t = nc.dram_tensor("h", [P, HCOLS], FP16, kind="ExternalInput")
    out_t = nc.dram_tensor("out", [R, SEQ], FP16, kind="ExternalOutput")
    h_ap = h_t.ap()
    out_ap = out_t.ap()

    with tile.TileContext(nc) as tc:
        with (
            tc.tile_pool(name="hpool", bufs=len(loads)) as hp,
            tc.tile_pool(name="opool", bufs=len(stores)) as op,
            tc.tile_pool(name="psum", bufs=8, space="PSUM") as psump,
        ):
            def emit_pass():
                # Phase 1: every load, back-to-back on the SP queue.
                ltiles = []  # (b0, nb, lead, ht)
                b0 = 0
                for li, nb in enumerate(loads):
                    lead = BLK if b0 == 0 else 0
                    dcol0 = BLK + P * b0 - lead
                    ht = hp.tile([P, lead + P * nb], FP16, tag="h", name=f"h{li}")
                    nc.sync.dma_start(ht[:], h_ap[:, dcol0 : BLK + P * (b0 + nb)])
                    ltiles.append((b0, nb, lead, ht))
                    b0 += nb
                btile = ltiles[0][3]  # band matrix = cols [0:122] of load 0

                def lhs_ap(blk):
                    """lhsT operand [128, 128] for block blk from its load tile."""
                    for tb0, tnb, tlead, ht in ltiles:
                        if tb0 <= blk < tb0 + tnb:
                            col = tlead + P * (blk - tb0)
                            return ht[:, col : col + P]
                    raise AssertionError(blk)

                # Phase 2: psum groups of <=4 blocks within each store span;
                # copy each group (plain 2D, natural column order — the host
                # deinterleaves) into the store's otile; flush per store.
                copy_i = 0
                sb0 = 0
                for si, sn in enumerate(stores):
                    c0 = BLK * sb0  # first output column of this store
                    w = min(BLK * (sb0 + sn), SEQ) - c0
                    otile = op.tile([P, w], FP16, tag="o", name=f"o{si}")
                    for g0 in range(0, sn, 4):
                        gn = min(4, sn - g0)
                        ps = psump.tile([P, BLK * 4], FP32, tag="ps")
                        for q in range(gn):
                            nc.tensor.matmul(
                                ps[:, BLK * q : BLK * (q + 1)],
                                lhs_ap(sb0 + g0 + q),
                                btile[:, 0:BLK],
                                start=True,
                                stop=True,
                            )
                        # real columns this group contributes (block 33 only
                        # has 70): contiguous in psum, so plain 2D copies.
                        gw = min(BLK * (sb0 + g0 + gn), SEQ) - BLK * (sb0 + g0)
                        loc0 = BLK * g0  # store-local col offset of group

                        def ccopy(a, b):
                            nonlocal copy_i
                            if copy_i % 2:
                                nc.scalar.copy(
                                    otile[:, loc0 + a : loc0 + b], ps[:, a:b]
                                )
                            else:
                                nc.vector.tensor_copy(
                                    otile[:, loc0 + a : loc0 + b], ps[:, a:b]
                                )
                            copy_i += 1

                        if o.get("split_copy", False) and gw > 256:
                            ccopy(0, gw // 2)
                            ccopy(gw // 2, gw)
                        else:
                            ccopy(0, gw)
                    sq = o.get("store_queues", ["sync"])
                    getattr(nc, sq[si % len(sq)]).dma_start(
                        out_ap[:, c0 : c0 + w], otile[:]
                    )
                    sb0 += sn

            if loop_n:
                with tc.For_i(0, loop_n, 1):
                    emit_pass()
            else:
                for _ in range(repeat):
                    emit_pass()

    # Note: instructions that end up with >1 sync wait (walrus encodes only
    # one on fp32 LDW+MM pairs etc.) are legalized by bacc's compile() below.
    nc.compile()
    return nc


def _get_nc(repeat=1, opts=None):
    key = ("nc", repeat, repr(sorted((opts or {}).items(), key=str)))
    if key not in _CACHE:
        _CACHE[key] = _build_bass(repeat, opts)
    return _CACHE[key]


def _pack_host(x, bmat):
    """Per-core input tensors: [band matrix | lhsT tiles], where lhsT tile i
    is x_shard.T[122i : 122i+128, :] (circularly padded). fp16 on the wire:
    rel err budget is 2e-2; fp16 quantization of x and the outputs lands at
    ~3e-4 while halving HBM traffic (memory-bound kernel)."""
    hs = []
    for c in range(N_CORES):
        xs = np.ascontiguousarray(x[R * c : R * (c + 1)].T)  # [4096, 128]
        xtp = np.concatenate([xs, xs[:P]], axis=0)            # circular pad
        H = np.empty((P, HCOLS), dtype=np.float16)
        H[:, 0:BLK] = bmat
        for i in range(NBLK):
            H[:, BLK + P * i : BLK + P * (i + 1)] = xtp[BLK * i : BLK * i + P]
        hs.append(H)
    return hs


def _band_matrix(W):
    """128x122 coefficient block (natural column order; the kernel emits
    y = x @ W in natural order and the host deinterleaves)."""
    return np.asarray(W, dtype=np.float32)[0:P, 0:BLK].astype(np.float16)


def run(x, W, trace=False):
    x = np.ascontiguousarray(np.asarray(x, dtype=np.float32))
    assert x.shape == (BATCH, SEQ), x.shape
    in_maps = [{"h": h} for h in _pack_host(x, _band_matrix(W))]
    res = run_bass_kernel_spmd(
        _get_nc(), in_maps, core_ids=list(range(N_CORES)), trace=trace
    )
    y = np.concatenate(
        [res.results[c]["out"].astype(np.float32) for c in range(N_CORES)], axis=0
    )
    # deinterleave: even cols = approximation, odd cols = detail
    out = np.concatenate([y[:, 0::2], y[:, 1::2]], axis=1)
    return out, res


def kernel(x, W):
    out, _ = run(x, W)
    return out

